# revision 22
# baseline (speedup 1.0000x reference)
"""Causal self-attention (B=4, T=2048, C=1024, H=16, D=64) on 8 trn2 cores.

Sharding: core i handles batch b = i % 4 and head-group g = i // 4
(8 heads per core).  Each core computes QKV for its heads, causal
attention, and a partial projection (w_proj rows for its heads).  The
host sums the two partial projections per batch.

v2 vs baseline:
  - x is transposed on the host (xt [C, T], bf16): no PE transposes.
  - All QKV inputs and w_proj in bf16 (half the DMA, same PE rate).
  - Causal narrowing: for diagonal key-tiles only the valid query
    columns are computed in scores / exp'd / AV'd / masked.
  - Score matmuls for the head pair (rows 0-63 vs 64-127) are emitted
    interleaved so the PE row-group concurrency overlaps them.
  - Denominator broadcast (K=1) matmuls col-paired (strips 0-1 vs 2-3).
  - DMA triggers spread over SP/Pool queues, keeping ACT free for exp.

Per-core pipeline, interleaved over Tq chunks c of 512 so the PE-heavy
QKV work of chunk c+1 fills the ACT-bound attention of chunk c.
"""
import sys

for _p in ("/opt/trn_rl_repo",):
    if _p not in sys.path:
        sys.path.insert(0, _p)

import numpy as np
import ml_dtypes

import concourse.bass as bass
import concourse.tile as tile
from concourse import bacc, mybir

F32 = mybir.dt.float32
F32R = mybir.dt.float32r
BF16 = mybir.dt.bfloat16
EXP = mybir.ActivationFunctionType.Exp

B, T, C = 4, 2048, 1024
H, D = 16, 64
HPC = 8              # heads per core
GD = HPC * D         # 512
NCORES = 8
TCH = 512            # Tq / T chunk width
NTCH = T // TCH      # 4
NKT = C // 128       # 8 contraction k-tiles over C
NTT = T // 128       # 16 T tiles

ATTN_DT = BF16       # dtype for Q^T/K^T/V/P^T (scores + AV matmuls)

import os
ABLATE = set(os.environ.get("KABLATE", "").split(","))  # timing-only ablations


def _declare_io(nc):
    return dict(
        xt=nc.dram_tensor("xt", [C, T], BF16, kind="ExternalInput").ap(),
        wqk=nc.dram_tensor("wqk", [C, 2 * GD], BF16, kind="ExternalInput").ap(),
        wv=nc.dram_tensor("wv", [C, GD], BF16, kind="ExternalInput").ap(),
        wp=nc.dram_tensor("wp", [GD, C], BF16, kind="ExternalInput").ap(),
        out=nc.dram_tensor("out", [T, C], F32, kind="ExternalOutput").ap(),
    )


def _build_attn(tc, io, rep=""):
    nc = tc.nc
    xt, wqk, wv, wp, out = io["xt"], io["wqk"], io["wv"], io["wp"], io["out"]

    from contextlib import ExitStack
    with ExitStack() as _es:
        constp = _es.enter_context(tc.tile_pool(name=f"const{rep}", bufs=1))
        qktp = _es.enter_context(tc.tile_pool(name=f"qkt{rep}", bufs=1))
        vextp = _es.enter_context(tc.tile_pool(name=f"vext{rep}", bufs=1))
        wqkp = _es.enter_context(tc.tile_pool(name=f"wqk{rep}", bufs=1))
        wvp = _es.enter_context(tc.tile_pool(name=f"wv{rep}", bufs=1))
        wpp = _es.enter_context(tc.tile_pool(name=f"wp{rep}", bufs=1))
        xtp = _es.enter_context(tc.tile_pool(name=f"xt{rep}", bufs=16))
        ytp = _es.enter_context(tc.tile_pool(name=f"yt{rep}", bufs=12))
        ptp = _es.enter_context(tc.tile_pool(name=f"pt{rep}", bufs=8))
        ysbp = _es.enter_context(tc.tile_pool(name=f"ysb{rep}", bufs=3))
        recp = _es.enter_context(tc.tile_pool(name=f"rec{rep}", bufs=2))
        osbp = _es.enter_context(tc.tile_pool(name=f"osb{rep}", bufs=2))
        # one merged 2-buf pool for QKV-filler / projection / bc matmul psum:
        # consecutive units alternate banks, so the PE never waits for the
        # DVE drain of the previous unit's bank
        gpj = _es.enter_context(tc.tile_pool(name=f"gpj_ps{rep}", bufs=2, space="PSUM"))
        sps = _es.enter_context(tc.tile_pool(name=f"s_ps{rep}", bufs=2, space="PSUM"))
        yps = _es.enter_context(tc.tile_pool(name=f"y_ps{rep}", bufs=2, space="PSUM"))

        # ---- constants
        ones = constp.tile([128, 64], F32, tag=f"ones{rep}")
        nc.gpsimd.memset(ones[:], 1.0)
        ones_r = constp.tile([128, 64], F32R, tag=f"ones_r{rep}")
        nc.vector.tensor_copy(ones_r[:], ones[:])
        cpt = None
        if "noexp" in ABLATE:
            cpt = constp.tile([128, 2 * TCH], ATTN_DT, tag=f"cpt{rep}")
            nc.gpsimd.memset(cpt[:], 0.001)

        # ---- persistent tensors
        qkt = [[qktp.tile([128, TCH], ATTN_DT, tag=f"qkt{m}_{cc}{rep}",
                          name=f"qkt{m}_{cc}{rep}") for cc in range(NTCH)]
               for m in range(8)]
        vext = [vextp.tile([128, HPC * 65], ATTN_DT, tag=f"vext{t}{rep}",
                           name=f"vext{t}{rep}") for t in range(NTT)]

        # ---- first x chunk + weight loads (x chunk 0 first so PE can start)
        def emit_x_dma(c, eng=None):
            x_sb = []
            for k in range(NKT):
                xr = xtp.tile([128, TCH], BF16, tag=f"xt{rep}",
                              name=f"xt{c}_{k}{rep}")
                e = eng if eng is not None else (nc.sync if k % 2 else nc.gpsimd)
                e.dma_start(xr[:], xt[bass.ts(k, 128), bass.ts(c, TCH)])
                x_sb.append(xr)
            return x_sb

        xt_store = {0: emit_x_dma(0, eng=nc.sync)}
        wqk_sb = [wqkp.tile([128, 2 * GD], BF16, tag=f"wqk{k}{rep}",
                            name=f"wqk{k}{rep}") for k in range(NKT)]
        # load the Q-columns of every k-tile first: the chunk-0 Q^T gemm can
        # then start while the K-columns are still streaming in
        for half in range(2):
            for k in range(NKT):
                eng = nc.gpsimd if k % 2 else nc.sync
                eng.dma_start(
                    wqk_sb[k][:, bass.ts(half, GD)],
                    wqk[bass.ts(k, 128), bass.ts(half, GD)],
                )
        wv_sb = [wvp.tile([128, GD], BF16, tag=f"wv{k}{rep}", name=f"wv{k}{rep}")
                 for k in range(NKT)]
        for k in range(NKT):
            eng = nc.gpsimd if k % 2 else nc.sync
            eng.dma_start(wv_sb[k][:], wv[bass.ts(k, 128), :])
        wp_sb = [wpp.tile([128, C], BF16, tag=f"wp{k}{rep}", name=f"wp{k}{rep}")
                 for k in range(4)]
        for k in range(4):
            nc.gpsimd.dma_start(wp_sb[k][:], wp[bass.ts(k, 128), :])
        ones8 = ones[:, 0:8].rearrange("p (h e) -> p h e", e=1)
        for t in range(NTT):
            nc.vector.tensor_copy(
                vext[t][:].rearrange("p (h e) -> p h e", e=65)[:, :, 64:65], ones8
            )

        # ---- main pipeline.  Per-engine execution follows emission order,
        # so QKV(c+1) / deferred projections are emitted as "filler" units
        # interleaved between attention steps of chunk c to keep the PE busy
        # while ACT works through the exps.
        from collections import deque

        # QKV units are emitted in TWO halves (4 contraction k-tiles each) so
        # filler pacing is finer-grained; the psum tile is carried across the
        # halves via qkv_open.
        qkv_open = {}

        def emit_qk_mtile(c, m, xt_sb, half=None):
            halves = (0, 1) if half is None else (half,)
            if 0 in halves:
                o_ps = gpj.tile([128, TCH], F32, tag=f"gpj{rep}", name=f"qk{c}_{m}{rep}")
                qkv_open[("qk", c, m)] = o_ps
            o_ps = qkv_open[("qk", c, m)]
            for hf in halves:
                for k in range(4 * hf, 4 * hf + 4):
                    nc.tensor.matmul(
                        out=o_ps[:],
                        lhsT=wqk_sb[k][:, bass.ts(m, 128)],
                        rhs=xt_sb[k][:],
                        start=(k == 0),
                        stop=(k == NKT - 1),
                    )
            if 1 in halves:
                nc.vector.tensor_copy(qkt[m][c][:], o_ps[:])
                del qkv_open[("qk", c, m)]

        def emit_v_jtile(c, j, xt_sb, half=None):
            halves = (0, 1) if half is None else (half,)
            if 0 in halves:
                o_ps = gpj.tile([128, GD], F32, tag=f"gpj{rep}", name=f"v{c}_{j}{rep}")
                qkv_open[("v", c, j)] = o_ps
            o_ps = qkv_open[("v", c, j)]
            for hf in halves:
                for k in range(4 * hf, 4 * hf + 4):
                    nc.tensor.matmul(
                        out=o_ps[:],
                        lhsT=xt_sb[k][:, bass.ts(j, 128)],
                        rhs=wv_sb[k][:],
                        start=(k == 0),
                        stop=(k == NKT - 1),
                    )
            if 1 in halves:
                dst = vext[c * 4 + j][:].rearrange("p (h e) -> p h e", e=65)
                nc.vector.tensor_copy(
                    dst[:, :, 0:64],
                    o_ps[:].rearrange("p (h e) -> p h e", e=64),
                )
                del qkv_open[("v", c, j)]

        def emit_proj_group(c, tt, n, yt_c, osb_acc={}):
            t = 4 * c + tt
            o_ps = gpj.tile([128, 512], F32, tag=f"gpj{rep}", name=f"pj{t}_{n}{rep}")
            for k in range(4):
                nc.tensor.matmul(
                    out=o_ps[:],
                    lhsT=yt_c[k][:, bass.ts(tt, 128)],
                    rhs=wp_sb[k][:, bass.ts(n, 512)],
                    start=(k == 0),
                    stop=(k == 3),
                )
            # pair the two half-rows into one osb tile and one 512KB store,
            # alternating DMA queues so the tail is not single-queue paced
            if n == 0:
                osb_acc[t] = osbp.tile([128, C], F32, tag=f"osb{rep}", name=f"osb{t}{rep}")
            osb = osb_acc[t]
            nc.vector.tensor_copy(osb[:, bass.ts(n, 512)], o_ps[:])
            if n == 1:
                eng = nc.sync if t % 2 else nc.gpsimd
                eng.dma_start(out[bass.ts(t, 128), :], osb[:])
                del osb_acc[t]

        def emit_att_pair(c, hp, yt_c, filler, stride=1, _tick=[0]):
            nr = 4 * c + 4
            heads = (2 * hp, 2 * hp + 1)
            qtile = qkt[hp]
            ktile = qkt[4 + hp]
            yext = {h: yps.tile([128, TCH], F32, tag=f"yext{rep}",
                                name=f"yext{h}_{c}{rep}") for h in heads}
            for r0 in range(0, nr, 2):
                s_ps = {h: sps.tile([128, 2 * TCH], F32, tag=f"s{rep}",
                                    name=f"s{c}_{h}_{r0}{rep}")
                        for h in heads}
                # offset of the first valid query column for key tile rr
                offs = {rr: max(0, 128 * (rr - 4 * c)) for rr in (r0, r0 + 1)}
                # scores: interleave the two heads (PE rows 0-63 vs 64-127)
                # per key tile so the row-group concurrency overlaps them
                for rr in (r0, r0 + 1):
                    for h in heads:
                        pr = 64 * (h % 2)
                        off = offs[rr]
                        nc.tensor.matmul(
                            out=s_ps[h][:, bass.ds((rr - r0) * TCH + off, TCH - off)],
                            lhsT=ktile[rr // 4][pr:pr + 64, bass.ts(rr % 4, 128)],
                            rhs=qtile[c][pr:pr + 64, bass.ds(off, TCH - off)],
                            start=True,
                            stop=True,
                        )
                pts = {}
                for h in heads:
                    if "noexp" in ABLATE:
                        pts[h] = cpt
                        continue
                    pt = ptp.tile([128, 2 * TCH], ATTN_DT, tag=f"pt{rep}",
                                  name=f"pt{c}_{h}_{r0}{rep}")
                    # one exp per 512-wide half: a single-bank PSUM read is
                    # ~6x faster per element than an AP spanning two banks
                    for rr in (r0, r0 + 1):
                        off = (rr - r0) * TCH + offs[rr]
                        w = TCH - offs[rr]
                        nc.scalar.activation(
                            pt[:, bass.ds(off, w)],
                            s_ps[h][:, bass.ds(off, w)], EXP, scale=0.125)
                    # triangular mask on the 128-wide diagonal blocks
                    for rr in (r0, r0 + 1):
                        j = rr - 4 * c
                        if 0 <= j <= 3:
                            off = (rr - r0) * TCH + 128 * j
                            nc.gpsimd.affine_select(
                                out=pt[:, bass.ds(off, 128)],
                                in_=pt[:, bass.ds(off, 128)],
                                compare_op=mybir.AluOpType.is_ge,
                                fill=0.0,
                                base=0,
                                pattern=[[1, 128]],
                                channel_multiplier=-1,
                            )
                    pts[h] = pt
                _tick[0] += 1
                if filler and _tick[0] % stride == 0:
                    filler.popleft()()
                for h in heads:
                    for rr in (r0, r0 + 1):
                        off = offs[rr]
                        nc.tensor.matmul(
                            out=yext[h][0:65, bass.ds(off, TCH - off)],
                            lhsT=vext[rr][:, h * 65:h * 65 + 65],
                            rhs=pts[h][:, bass.ds((rr - r0) * TCH + off, TCH - off)],
                            start=(rr == 0),
                            stop=(rr == nr - 1),
                            skip_group_check=True,
                        )
                _tick[0] += 1
                if filler and _tick[0] % stride == 0:
                    filler.popleft()()
            # normalization: rec (DVE) -> bc broadcast (PE K=1) -> mul
            for h in heads:
                pr = 64 * (h % 2)
                ysb = ysbp.tile([128, TCH], F32, tag=f"ysb{rep}", name=f"ysb{h}_{c}{rep}")
                nc.vector.tensor_copy(ysb[0:65, :], yext[h][0:65, :])
                rec = recp.tile([128, TCH], F32R, tag=f"rec{rep}")
                with nc.allow_low_precision(reason="f32r denominators"):
                    nc.vector.reciprocal(rec[64:65, :], ysb[64:65, :])
                # reuse the just-freed yext psum slot for the broadcast
                bc = yps.tile([64, TCH], F32, tag=f"yext{rep}", name=f"bc{h}_{c}{rep}")
                nc.tensor.matmul(
                    out=bc[:],
                    lhsT=ones_r[64:65, :],
                    rhs=rec[64:65, :],
                    start=True,
                    stop=True,
                )
                nc.vector.tensor_mul(
                    yt_c[hp][pr:pr + 64, :],
                    ysb[0:64, :],
                    bc[:],
                )

        # QKV(0) emitted directly (nothing to interleave into)
        for m in range(8):
            emit_qk_mtile(0, m, xt_store[0])
        for j in range(4):
            emit_v_jtile(0, j, xt_store[0])

        yt_store = {}
        for c in range(NTCH):
            yt_c = [ytp.tile([128, TCH], ATTN_DT, tag=f"yt{rep}", name=f"yt{c}_{k}{rep}")
                    for k in range(4)]
            yt_store[c] = yt_c
            filler = deque()
            if c < 3:
                cn = c + 1
                xt_store[cn] = emit_x_dma(cn)
                mlist = range(8) if cn < 3 else range(5)   # chunk 3: Q part + K m4
                for m in mlist:
                    for hf in range(2):
                        filler.append(lambda cn=cn, m=m, hf=hf:
                                      emit_qk_mtile(cn, m, xt_store[cn], half=hf))
                for j in range(4):
                    for hf in range(2):
                        filler.append(lambda cn=cn, j=j, hf=hf:
                                      emit_v_jtile(cn, j, xt_store[cn], half=hf))
                if c == 1:
                    for tt in range(4):
                        for n in range(2):
                            filler.append(lambda tt=tt, n=n: emit_proj_group(0, tt, n, yt_store[0]))
                if c == 2:
                    for tt in range(4):
                        for n in range(2):
                            filler.append(lambda tt=tt, n=n: emit_proj_group(1, tt, n, yt_store[1]))
            else:
                for tt in range(4):
                    for n in range(2):
                        filler.append(lambda tt=tt, n=n: emit_proj_group(2, tt, n, yt_store[2]))
            stride = {0: 1, 1: 1, 2: 2, 3: 8}[c]   # ~ticks / fillers
            for hp in range(HPC // 2):
                if c == 3 and hp > 0:   # K^T tile for this head pair
                    emit_qk_mtile(3, 4 + hp, xt_store[3])
                emit_att_pair(c, hp, yt_c, filler, stride=stride)
            while filler:
                filler.popleft()()
        for tt in range(4):
            for n in range(2):
                emit_proj_group(3, tt, n, yt_store[3])


_NC_CACHE = None


def _get_nc(reps=1, loop=0):
    """reps: unrolled body copies; loop: hardware For_i wrap (timing only)."""
    global _NC_CACHE
    key = (reps, loop)
    if _NC_CACHE is None or _NC_CACHE[0] != key:
        nc = bacc.Bacc("TRN2", target_bir_lowering=False, debug=False,
                       num_devices=NCORES)
        with tile.TileContext(nc, trace_sim=False) as tc:
            io = _declare_io(nc)
            if loop:
                with tc.For_i(0, loop, 1):
                    _build_attn(tc, io)
            else:
                for r in range(reps):
                    _build_attn(tc, io, rep="" if reps == 1 else f"_r{r}")
        nc.compile()
        _NC_CACHE = (key, nc)
    return _NC_CACHE[1]


def shard_inputs(x, w_qkv, w_proj):
    """Build the 8 per-core input maps (host-side transpose + bf16 cast)."""
    bf16 = ml_dtypes.bfloat16
    in_maps = []
    for i in range(NCORES):
        b, g = i % B, i // B
        cols = slice(g * GD, (g + 1) * GD)
        in_maps.append({
            "xt": np.ascontiguousarray(x[b].T).astype(bf16),
            "wqk": np.ascontiguousarray(
                np.concatenate([w_qkv[:, 0 * C:][:, cols], w_qkv[:, 1 * C:][:, cols]], axis=1)
            ).astype(bf16),
            "wv": np.ascontiguousarray(w_qkv[:, 2 * C:][:, cols]).astype(bf16),
            "wp": np.ascontiguousarray(w_proj[g * GD:(g + 1) * GD, :]).astype(bf16),
        })
    return in_maps


def unshard_output(results):
    out = np.empty((B, T, C), dtype=np.float32)
    for b in range(B):
        out[b] = results[b]["out"] + results[b + B]["out"]
    return out


def kernel(x, w_qkv, w_proj):
    from concourse.bass_utils import run_bass_kernel_spmd
    x = np.asarray(x, dtype=np.float32)
    w_qkv = np.asarray(w_qkv, dtype=np.float32)
    w_proj = np.asarray(w_proj, dtype=np.float32)
    nc = _get_nc()
    in_maps = shard_inputs(x, w_qkv, w_proj)
    res = run_bass_kernel_spmd(nc, in_maps, list(range(NCORES)))
    return unshard_output(res.results)


# revision 27
# speedup vs baseline: 1.1859x; 1.1859x over previous
"""Causal self-attention (B=4, T=2048, C=1024, H=16, D=64) on 8 trn2 cores.

Sharding: core i handles batch b = i % 4 and head-group g = i // 4
(8 heads per core).  Each core computes QKV for its heads, causal
attention, and a partial projection (w_proj rows for its heads).  The
host sums the two partial projections per batch.

v2 vs baseline:
  - x is transposed on the host (xt [C, T], bf16): no PE transposes.
  - All QKV inputs and w_proj in bf16 (half the DMA, same PE rate).
  - Causal narrowing: for diagonal key-tiles only the valid query
    columns are computed in scores / exp'd / AV'd / masked.
  - Score matmuls for the head pair (rows 0-63 vs 64-127) are emitted
    interleaved so the PE row-group concurrency overlaps them.
  - Denominator broadcast (K=1) matmuls col-paired (strips 0-1 vs 2-3).
  - DMA triggers spread over SP/Pool queues, keeping ACT free for exp.

Per-core pipeline, interleaved over Tq chunks c of 512 so the PE-heavy
QKV work of chunk c+1 fills the ACT-bound attention of chunk c.
"""
import sys

for _p in ("/opt/trn_rl_repo",):
    if _p not in sys.path:
        sys.path.insert(0, _p)

import numpy as np
import ml_dtypes

import concourse.bass as bass
import concourse.tile as tile
from concourse import bacc, mybir

F32 = mybir.dt.float32
F32R = mybir.dt.float32r
BF16 = mybir.dt.bfloat16
EXP = mybir.ActivationFunctionType.Exp

B, T, C = 4, 2048, 1024
H, D = 16, 64
HPC = 8              # heads per core
GD = HPC * D         # 512
NCORES = 8
TCH = 512            # Tq / T chunk width
NTCH = T // TCH      # 4
NKT = C // 128       # 8 contraction k-tiles over C
NTT = T // 128       # 16 T tiles

ATTN_DT = BF16       # dtype for Q^T/K^T/V/P^T (scores + AV matmuls)

import os
ABLATE = set(os.environ.get("KABLATE", "").split(","))  # timing-only ablations


def _declare_io(nc):
    return dict(
        xt=nc.dram_tensor("xt", [C, T], BF16, kind="ExternalInput").ap(),
        wqk=nc.dram_tensor("wqk", [C, 2 * GD], BF16, kind="ExternalInput").ap(),
        wv=nc.dram_tensor("wv", [C, GD], BF16, kind="ExternalInput").ap(),
        wp=nc.dram_tensor("wp", [GD, C], BF16, kind="ExternalInput").ap(),
        out=nc.dram_tensor("out", [T, C], F32, kind="ExternalOutput").ap(),
    )


def _build_attn(tc, io, rep=""):
    nc = tc.nc
    xt, wqk, wv, wp, out = io["xt"], io["wqk"], io["wv"], io["wp"], io["out"]

    from contextlib import ExitStack
    with ExitStack() as _es:
        constp = _es.enter_context(tc.tile_pool(name=f"const{rep}", bufs=1))
        qktp = _es.enter_context(tc.tile_pool(name=f"qkt{rep}", bufs=1))
        vextp = _es.enter_context(tc.tile_pool(name=f"vext{rep}", bufs=1))
        wqkp = _es.enter_context(tc.tile_pool(name=f"wqk{rep}", bufs=1))
        wvp = _es.enter_context(tc.tile_pool(name=f"wv{rep}", bufs=1))
        wpp = _es.enter_context(tc.tile_pool(name=f"wp{rep}", bufs=1))
        xtp = _es.enter_context(tc.tile_pool(name=f"xt{rep}", bufs=16))
        ytp = _es.enter_context(tc.tile_pool(name=f"yt{rep}", bufs=12))
        ptp = _es.enter_context(tc.tile_pool(name=f"pt{rep}", bufs=8))
        ysbp = _es.enter_context(tc.tile_pool(name=f"ysb{rep}", bufs=3))
        recp = _es.enter_context(tc.tile_pool(name=f"rec{rep}", bufs=2))
        osbp = _es.enter_context(tc.tile_pool(name=f"osb{rep}", bufs=2))
        # one merged 2-buf pool for QKV-filler / projection / bc matmul psum:
        # consecutive units alternate banks, so the PE never waits for the
        # DVE drain of the previous unit's bank
        gpj = _es.enter_context(tc.tile_pool(name=f"gpj_ps{rep}", bufs=2, space="PSUM"))
        # single-bank score tiles (cross-bank ACT reads are ~6x slower);
        # 4 bufs = two (head-pair, key-tile) steps in flight
        sps = _es.enter_context(tc.tile_pool(name=f"s_ps{rep}", bufs=4, space="PSUM"))
        yps = _es.enter_context(tc.tile_pool(name=f"y_ps{rep}", bufs=2, space="PSUM"))

        # ---- constants
        ones = constp.tile([128, 64], F32, tag=f"ones{rep}")
        nc.gpsimd.memset(ones[:], 1.0)
        ones_r = constp.tile([128, 64], F32R, tag=f"ones_r{rep}")
        nc.vector.tensor_copy(ones_r[:], ones[:])
        cpt = None
        if "noexp" in ABLATE:
            cpt = constp.tile([128, 2 * TCH], ATTN_DT, tag=f"cpt{rep}")
            nc.gpsimd.memset(cpt[:], 0.001)

        # ---- persistent tensors
        qkt = [[qktp.tile([128, TCH], ATTN_DT, tag=f"qkt{m}_{cc}{rep}",
                          name=f"qkt{m}_{cc}{rep}") for cc in range(NTCH)]
               for m in range(8)]
        vext = [vextp.tile([128, HPC * 65], ATTN_DT, tag=f"vext{t}{rep}",
                           name=f"vext{t}{rep}") for t in range(NTT)]

        # ---- first x chunk + weight loads (x chunk 0 first so PE can start)
        def emit_x_dma(c, eng=None):
            x_sb = []
            for k in range(NKT):
                xr = xtp.tile([128, TCH], BF16, tag=f"xt{rep}",
                              name=f"xt{c}_{k}{rep}")
                e = eng if eng is not None else (nc.sync if k % 2 else nc.gpsimd)
                e.dma_start(xr[:], xt[bass.ts(k, 128), bass.ts(c, TCH)])
                x_sb.append(xr)
            return x_sb

        xt_store = {0: emit_x_dma(0, eng=nc.sync)}
        wqk_sb = [wqkp.tile([128, 2 * GD], BF16, tag=f"wqk{k}{rep}",
                            name=f"wqk{k}{rep}") for k in range(NKT)]
        # load the Q-columns of every k-tile first: the chunk-0 Q^T gemm can
        # then start while the K-columns are still streaming in
        for half in range(2):
            for k in range(NKT):
                eng = nc.gpsimd if k % 2 else nc.sync
                eng.dma_start(
                    wqk_sb[k][:, bass.ts(half, GD)],
                    wqk[bass.ts(k, 128), bass.ts(half, GD)],
                )
        wv_sb = [wvp.tile([128, GD], BF16, tag=f"wv{k}{rep}", name=f"wv{k}{rep}")
                 for k in range(NKT)]
        for k in range(NKT):
            eng = nc.gpsimd if k % 2 else nc.sync
            eng.dma_start(wv_sb[k][:], wv[bass.ts(k, 128), :])
        wp_sb = [wpp.tile([128, C], BF16, tag=f"wp{k}{rep}", name=f"wp{k}{rep}")
                 for k in range(4)]
        for k in range(4):
            nc.gpsimd.dma_start(wp_sb[k][:], wp[bass.ts(k, 128), :])
        ones8 = ones[:, 0:8].rearrange("p (h e) -> p h e", e=1)
        for t in range(NTT):
            nc.vector.tensor_copy(
                vext[t][:].rearrange("p (h e) -> p h e", e=65)[:, :, 64:65], ones8
            )

        # ---- main pipeline.  Per-engine execution follows emission order,
        # so QKV(c+1) / deferred projections are emitted as "filler" units
        # interleaved between attention steps of chunk c to keep the PE busy
        # while ACT works through the exps.
        from collections import deque

        # QKV units are emitted in TWO halves (4 contraction k-tiles each) so
        # filler pacing is finer-grained; the psum tile is carried across the
        # halves via qkv_open.
        qkv_open = {}

        def emit_qk_mtile(c, m, xt_sb, half=None):
            halves = (0, 1) if half is None else (half,)
            if 0 in halves:
                o_ps = gpj.tile([128, TCH], F32, tag=f"gpj{rep}", name=f"qk{c}_{m}{rep}")
                qkv_open[("qk", c, m)] = o_ps
            o_ps = qkv_open[("qk", c, m)]
            for hf in halves:
                for k in range(4 * hf, 4 * hf + 4):
                    for _ in range(2 if "qkv2x" in ABLATE else 1):
                        nc.tensor.matmul(
                            out=o_ps[:],
                            lhsT=wqk_sb[k][:, bass.ts(m, 128)],
                            rhs=xt_sb[k][:],
                            start=(k == 0),
                            stop=(k == NKT - 1),
                        )
            if 1 in halves:
                nc.vector.tensor_copy(qkt[m][c][:], o_ps[:])
                del qkv_open[("qk", c, m)]

        def emit_v_jtile(c, j, xt_sb, half=None):
            halves = (0, 1) if half is None else (half,)
            if 0 in halves:
                o_ps = gpj.tile([128, GD], F32, tag=f"gpj{rep}", name=f"v{c}_{j}{rep}")
                qkv_open[("v", c, j)] = o_ps
            o_ps = qkv_open[("v", c, j)]
            for hf in halves:
                for k in range(4 * hf, 4 * hf + 4):
                    nc.tensor.matmul(
                        out=o_ps[:],
                        lhsT=xt_sb[k][:, bass.ts(j, 128)],
                        rhs=wv_sb[k][:],
                        start=(k == 0),
                        stop=(k == NKT - 1),
                    )
            if 1 in halves:
                dst = vext[c * 4 + j][:].rearrange("p (h e) -> p h e", e=65)
                nc.vector.tensor_copy(
                    dst[:, :, 0:64],
                    o_ps[:].rearrange("p (h e) -> p h e", e=64),
                )
                del qkv_open[("v", c, j)]

        def emit_proj_group(c, tt, n, yt_c, osb_acc={}):
            t = 4 * c + tt
            o_ps = gpj.tile([128, 512], F32, tag=f"gpj{rep}", name=f"pj{t}_{n}{rep}")
            for k in range(4):
                nc.tensor.matmul(
                    out=o_ps[:],
                    lhsT=yt_c[k][:, bass.ts(tt, 128)],
                    rhs=wp_sb[k][:, bass.ts(n, 512)],
                    start=(k == 0),
                    stop=(k == 3),
                )
            # pair the two half-rows into one osb tile and one 512KB store,
            # alternating DMA queues so the tail is not single-queue paced
            if n == 0:
                osb_acc[t] = osbp.tile([128, C], F32, tag=f"osb{rep}", name=f"osb{t}{rep}")
            osb = osb_acc[t]
            nc.vector.tensor_copy(osb[:, bass.ts(n, 512)], o_ps[:])
            if n == 1:
                eng = nc.sync if t % 2 else nc.gpsimd
                eng.dma_start(out[bass.ts(t, 128), :], osb[:])
                del osb_acc[t]

        def emit_att_pair(c, hp, yt_c, filler, stride=1, _tick=[0]):
            nr = 4 * c + 4
            heads = (2 * hp, 2 * hp + 1)
            qtile = qkt[hp]
            ktile = qkt[4 + hp]
            yext = {h: yps.tile([128, TCH], F32, tag=f"yext{rep}",
                                name=f"yext{h}_{c}{rep}") for h in heads}
            for r0 in range(0, nr, 2):
                s_ps = {h: sps.tile([128, 2 * TCH], F32, tag=f"s{rep}",
                                    name=f"s{c}_{h}_{r0}{rep}")
                        for h in heads}
                # offset of the first valid query column for key tile rr
                offs = {rr: max(0, 128 * (rr - 4 * c)) for rr in (r0, r0 + 1)}
                # scores: interleave the two heads (PE rows 0-63 vs 64-127)
                # per key tile so the row-group concurrency overlaps them
                for rr in (r0, r0 + 1):
                    for h in heads:
                        pr = 64 * (h % 2)
                        off = offs[rr]
                        for _ in range(2 if "scores2x" in ABLATE else 1):
                            nc.tensor.matmul(
                                out=s_ps[h][:, bass.ds((rr - r0) * TCH + off, TCH - off)],
                                lhsT=ktile[rr // 4][pr:pr + 64, bass.ts(rr % 4, 128)],
                                rhs=qtile[c][pr:pr + 64, bass.ds(off, TCH - off)],
                                start=True,
                                stop=True,
                            )
                pts = {}
                for h in heads:
                    if "noexp" in ABLATE:
                        pts[h] = cpt
                        continue
                    pt = ptp.tile([128, 2 * TCH], ATTN_DT, tag=f"pt{rep}",
                                  name=f"pt{c}_{h}_{r0}{rep}")
                    # one exp per 512-wide half: a single-bank PSUM read is
                    # ~6x faster per element than an AP spanning two banks
                    for rr in (r0, r0 + 1):
                        off = (rr - r0) * TCH + offs[rr]
                        w = TCH - offs[rr]
                        for _ in range(2 if "exp2x" in ABLATE else 1):
                            nc.scalar.activation(
                                pt[:, bass.ds(off, w)],
                                s_ps[h][:, bass.ds(off, w)], EXP, scale=0.125)
                    # triangular mask on the 128-wide diagonal blocks
                    for rr in (r0, r0 + 1):
                        j = rr - 4 * c
                        if 0 <= j <= 3:
                            off = (rr - r0) * TCH + 128 * j
                            nc.gpsimd.affine_select(
                                out=pt[:, bass.ds(off, 128)],
                                in_=pt[:, bass.ds(off, 128)],
                                compare_op=mybir.AluOpType.is_ge,
                                fill=0.0,
                                base=0,
                                pattern=[[1, 128]],
                                channel_multiplier=-1,
                            )
                    pts[h] = pt
                _tick[0] += 1
                if filler and _tick[0] % stride == 0:
                    filler.popleft()()
                for h in heads:
                    for rr in (r0, r0 + 1):
                        off = offs[rr]
                        for _ in range(2 if "av2x" in ABLATE else 1):
                            nc.tensor.matmul(
                                out=yext[h][0:65, bass.ds(off, TCH - off)],
                                lhsT=vext[rr][:, h * 65:h * 65 + 65],
                                rhs=pts[h][:, bass.ds((rr - r0) * TCH + off, TCH - off)],
                                start=(rr == 0),
                                stop=(rr == nr - 1),
                                skip_group_check=True,
                            )
                _tick[0] += 1
                if filler and _tick[0] % stride == 0:
                    filler.popleft()()
            # normalization: rec (DVE) -> bc broadcast (PE K=1) -> mul
            for h in heads:
                pr = 64 * (h % 2)
                ysb = ysbp.tile([128, TCH], F32, tag=f"ysb{rep}", name=f"ysb{h}_{c}{rep}")
                nc.vector.tensor_copy(ysb[0:65, :], yext[h][0:65, :])
                rec = recp.tile([128, TCH], F32R, tag=f"rec{rep}")
                with nc.allow_low_precision(reason="f32r denominators"):
                    nc.vector.reciprocal(rec[64:65, :], ysb[64:65, :])
                # reuse the just-freed yext psum slot for the broadcast
                bc = yps.tile([64, TCH], F32, tag=f"yext{rep}", name=f"bc{h}_{c}{rep}")
                nc.tensor.matmul(
                    out=bc[:],
                    lhsT=ones_r[64:65, :],
                    rhs=rec[64:65, :],
                    start=True,
                    stop=True,
                )
                nc.vector.tensor_mul(
                    yt_c[hp][pr:pr + 64, :],
                    ysb[0:64, :],
                    bc[:],
                )

        # QKV(0) emitted directly (nothing to interleave into)
        for m in range(8):
            emit_qk_mtile(0, m, xt_store[0])
        for j in range(4):
            emit_v_jtile(0, j, xt_store[0])

        yt_store = {}
        for c in range(NTCH):
            yt_c = [ytp.tile([128, TCH], ATTN_DT, tag=f"yt{rep}", name=f"yt{c}_{k}{rep}")
                    for k in range(4)]
            yt_store[c] = yt_c
            filler = deque()
            if c < 3:
                cn = c + 1
                xt_store[cn] = emit_x_dma(cn)
                mlist = range(8) if cn < 3 else range(5)   # chunk 3: Q part + K m4
                for m in mlist:
                    for hf in range(2):
                        filler.append(lambda cn=cn, m=m, hf=hf:
                                      emit_qk_mtile(cn, m, xt_store[cn], half=hf))
                for j in range(4):
                    for hf in range(2):
                        filler.append(lambda cn=cn, j=j, hf=hf:
                                      emit_v_jtile(cn, j, xt_store[cn], half=hf))
                if c == 1:
                    for tt in range(4):
                        for n in range(2):
                            filler.append(lambda tt=tt, n=n: emit_proj_group(0, tt, n, yt_store[0]))
                if c == 2:
                    for tt in range(4):
                        for n in range(2):
                            filler.append(lambda tt=tt, n=n: emit_proj_group(1, tt, n, yt_store[1]))
            else:
                for tt in range(4):
                    for n in range(2):
                        filler.append(lambda tt=tt, n=n: emit_proj_group(2, tt, n, yt_store[2]))
            stride = {0: 1, 1: 1, 2: 2, 3: 8}[c]   # ~ticks / fillers
            for hp in range(HPC // 2):
                if c == 3 and hp > 0:   # K^T tile for this head pair
                    emit_qk_mtile(3, 4 + hp, xt_store[3])
                emit_att_pair(c, hp, yt_c, filler, stride=stride)
            while filler:
                filler.popleft()()
        for tt in range(4):
            for n in range(2):
                emit_proj_group(3, tt, n, yt_store[3])


_NC_CACHE = None


def _get_nc(reps=1, loop=0):
    """reps: unrolled body copies; loop: hardware For_i wrap (timing only)."""
    global _NC_CACHE
    key = (reps, loop)
    if _NC_CACHE is None or _NC_CACHE[0] != key:
        nc = bacc.Bacc("TRN2", target_bir_lowering=False, debug=False,
                       num_devices=NCORES)
        with tile.TileContext(nc, trace_sim=False) as tc:
            io = _declare_io(nc)
            if loop:
                with tc.For_i(0, loop, 1):
                    _build_attn(tc, io)
            else:
                for r in range(reps):
                    _build_attn(tc, io, rep="" if reps == 1 else f"_r{r}")
        nc.compile()
        _NC_CACHE = (key, nc)
    return _NC_CACHE[1]


def shard_inputs(x, w_qkv, w_proj):
    """Build the 8 per-core input maps (host-side transpose + bf16 cast)."""
    bf16 = ml_dtypes.bfloat16
    in_maps = []
    for i in range(NCORES):
        b, g = i % B, i // B
        cols = slice(g * GD, (g + 1) * GD)
        in_maps.append({
            "xt": np.ascontiguousarray(x[b].T).astype(bf16),
            "wqk": np.ascontiguousarray(
                np.concatenate([w_qkv[:, 0 * C:][:, cols], w_qkv[:, 1 * C:][:, cols]], axis=1)
            ).astype(bf16),
            "wv": np.ascontiguousarray(w_qkv[:, 2 * C:][:, cols]).astype(bf16),
            "wp": np.ascontiguousarray(w_proj[g * GD:(g + 1) * GD, :]).astype(bf16),
        })
    return in_maps


def unshard_output(results):
    out = np.empty((B, T, C), dtype=np.float32)
    for b in range(B):
        out[b] = results[b]["out"] + results[b + B]["out"]
    return out


def kernel(x, w_qkv, w_proj):
    from concourse.bass_utils import run_bass_kernel_spmd
    x = np.asarray(x, dtype=np.float32)
    w_qkv = np.asarray(w_qkv, dtype=np.float32)
    w_proj = np.asarray(w_proj, dtype=np.float32)
    nc = _get_nc()
    in_maps = shard_inputs(x, w_qkv, w_proj)
    res = run_bass_kernel_spmd(nc, in_maps, list(range(NCORES)))
    return unshard_output(res.results)
